# revision 4
# baseline (speedup 1.0000x reference)
"""nn_Attention multi-head attention on 8 TRN2 NeuronCores.

Sharding: core c handles batch b=c//2 and query-half qh=c%2 (1024 query
tokens). QKV projections run only over the core's OWN 1024 tokens; the
K^T/V halves are exchanged between the two cores of a batch with a
pairwise HBM AllGather (replica groups {2b, 2b+1}), so no projection
work is duplicated. Keys are kept in global batch order (even core's
tokens first) on both cores, so the exchange read-back is
program-uniform. The host concatenates the 8 disjoint [1024, 1024]
output slices.

Device-side structure (per core, per head-pair p):
  - w_qkv slices loaded with one DMA per contraction tile from a
    host-reordered [c, (pair, m, d)] layout
  - K^T/V'/Q^T computed for own tokens; V' obtained with full [128,128]
    PE transposes (both heads at once) into 65-wide V-slots whose last
    column is 1.0 (softmax denominator via the ones column)
  - own K^T/V' halves staged to an HBM bounce, AllGather'd across the
    core pair, and read back into the full-key kt_sb / vp buffers
  - attention in transposed layout S^T = K_h Q_h^T per 128-key tile;
    exp on the Scalar engine straight out of PSUM; all matmuls bf16
    with fp32 PSUM
  - the QKV+exchange of the NEXT pair and the output-projection
    partials of PREVIOUS pairs (batched two pairs per PSUM
    accumulation group, bias folded into the first batch) are
    emission-interleaved into the attention key-tile loop
"""

import contextlib

import numpy as np
import orjson

import concourse.bass as bass
import concourse.mybir as mybir
import concourse.tile as tile
from concourse.vector_clock import ScopedClock

# ---------------------------------------------------------------------------
# Workarounds for the walrus build in this container, which accepts at most
# one sync wait per engine instruction (two for EventSemaphore):
#  1. Tile's end-of-kernel drain carries one wait per outstanding semaphore --
#     redistribute over a chain of sync-engine NOPs.
#  2. Tile's scheduler also emits multi-wait body instructions -- split them
#     in the serialized BIR by inserting same-engine NOPs ahead of the
#     offender (engine program order makes the chain equivalent).
# ---------------------------------------------------------------------------


def _patched_drain_and_barrier(self, tick_clock, wait_clock):
    nc = self.nc
    collector = nc.sync.nop()
    wait_clock.add_sem_waits(
        collector.ins, ScopedClock({None: tick_clock.global_clock})
    )
    si = collector.ins.sync_info
    waits = list(si.on_wait or []) if si is not None else []
    if si is not None:
        si.on_wait = waits[:1]
    import bass_rust as _br

    for w in waits[1:]:
        n = nc.sync.nop()
        n.ins.sync_info = _br.SyncInfo(on_wait=[w], on_update=[])

    nc.sync.drain()
    nc.all_engine_barrier()
    assert self.sems is not None
    popped = nc._tile_sem_poison_stack.pop()
    assert popped is self._sem_poison
    nc.clear_and_free_semaphores(list(self.sems.allocated().values()))
    nc.all_engine_barrier()


_WCAPS = {"EventSemaphore": 2}
_wcounter = [0]


def _split_waits_json(bir_bytes: bytes) -> bytes:
    j = orjson.loads(bir_bytes)
    changed_any = False
    for f in j.get("functions", []):
        for b in f.get("blocks", []):
            outl = []
            changed = False
            for ins in b["instructions"]:
                si = ins.get("sync_info")
                waits = (si or {}).get("on_wait") or []
                cap = _WCAPS.get(ins.get("opcode"), 1)
                engine = ins.get("engine")
                if len(waits) > cap and engine and engine != "Unassigned":
                    changed = True
                    extra, keep = waits[:-cap], waits[-cap:]
                    for w in extra:
                        _wcounter[0] += 1
                        outl.append({
                            "name": f"I-wsplit-{_wcounter[0]}",
                            "opcode": "NoOp",
                            "engine": engine,
                            "ins": [],
                            "outs": [],
                            "sync_info": {"on_update": [], "on_wait": [w]},
                        })
                    si["on_wait"] = keep
                outl.append(ins)
            if changed:
                b["instructions"] = outl
                changed_any = True
    return orjson.dumps(j) if changed_any else bir_bytes


def _apply_patches():
    if not getattr(tile.TileContext, "_attn_drain_patched", False):
        tile.TileContext._drain_and_barrier = _patched_drain_and_barrier
        tile.TileContext._attn_drain_patched = True
    if not getattr(bass.Bass, "_attn_wait_split_patched", False):
        orig = bass.Bass.to_json_bytes

        def to_json_bytes(self, *a, **kw):
            return _split_waits_json(orig(self, *a, **kw))

        bass.Bass.to_json_bytes = to_json_bytes
        bass.Bass._attn_wait_split_patched = True


F32 = mybir.dt.float32
BF16 = mybir.dt.bfloat16

C = 1024
H = 16
HD = 64
NK = 2048
NQ = 1024
SCALE = HD ** -0.5
KT_TILES = NK // 128   # 16 key tiles (full)
KT_OWN = NQ // 128     # 8 key tiles computed locally
CT_TILES = C // 128
VSLOT = 65             # 64 v dims + ones column
SLOTW = 2 * VSLOT      # both heads of a pair per key tile
NPAIR = H // 2
BOUNCE_W = NQ + KT_OWN * SLOTW  # 1024 K cols + 1040 V cols


def build_nc():
    _apply_patches()
    nc = bass.Bass("TRN2", num_devices=8)
    xt = nc.declare_dram_parameter("xt", [C, NQ], BF16, isOutput=False)
    wqkvt = nc.declare_dram_parameter("wqkvt", [C, 3 * C], BF16, isOutput=False)
    wpt = nc.declare_dram_parameter("wpt", [C, C], BF16, isOutput=False)
    bias = nc.declare_dram_parameter("bias", [1, C], BF16, isOutput=False)
    out = nc.declare_dram_parameter("out", [NQ, C], F32, isOutput=True)

    groups = [[0, 1], [2, 3], [4, 5], [6, 7]]

    with tile.TileContext(nc) as tc:
        with contextlib.ExitStack() as es:
            persist = es.enter_context(tc.tile_pool(name="persist", bufs=1))
            ones = persist.tile([1, 128], BF16, tag="ones")
            nc.vector.memset(ones[:], 1.0)
            ident = persist.tile([128, 128], BF16, tag="ident")
            nc.gpsimd.memset(ident[:], 0.0)
            nc.gpsimd.affine_select(
                out=ident[:], in_=ident[:],
                compare_op=mybir.AluOpType.not_equal, fill=1.0,
                base=0, pattern=[[-1, 128]], channel_multiplier=1,
            )
            bias_sb = persist.tile([1, C], BF16, tag="bias")
            nc.sync.dma_start(out=bias_sb[:], in_=bias[:])

            nts = [persist.tile([128, NQ], BF16, tag=f"nt{ct}", name=f"nt{ct}")
                   for ct in range(CT_TILES)]
            vprimes = [persist.tile([128, KT_TILES * SLOTW], BF16,
                                    tag=f"vp{i}", name=f"vp{i}") for i in range(2)]
            oaccs = [persist.tile([128, C], F32, tag=f"oa{tt}", name=f"oa{tt}")
                     for tt in range(NQ // 128)]
            wp_pool = es.enter_context(tc.tile_pool(name="wp", bufs=CT_TILES))
            wpts = [wp_pool.tile([128, C], BF16, tag="wpt", name=f"wpts{ct}")
                    for ct in range(CT_TILES)]

            dram = es.enter_context(tc.tile_pool(name="dram", bufs=1, space="DRAM"))
            b_ins = [dram.tile([128, BOUNCE_W], BF16, tag=f"bi{p}", name=f"bi{p}")
                     for p in range(NPAIR)]
            b_outs = [dram.tile([2, 128, BOUNCE_W], BF16, tag=f"bo{p}",
                                name=f"bo{p}") for p in range(NPAIR)]

            psum_mm = es.enter_context(tc.tile_pool(name="psum_mm", bufs=2, space="PSUM"))
            psum_s = es.enter_context(tc.tile_pool(name="psum_s", bufs=2, space="PSUM"))
            psum_u = es.enter_context(tc.tile_pool(name="psum_u", bufs=1, space="PSUM"))

            with contextlib.ExitStack() as es_attn:
                xt_pool = es_attn.enter_context(tc.tile_pool(name="xtp", bufs=CT_TILES))
                wq_pool = es_attn.enter_context(tc.tile_pool(name="wq", bufs=2))
                kt_pool = es_attn.enter_context(tc.tile_pool(name="kt", bufs=2))
                qt_pool = es_attn.enter_context(tc.tile_pool(name="qt", bufs=2))
                ko_pool = es_attn.enter_context(tc.tile_pool(name="ko", bufs=2))
                vo_pool = es_attn.enter_context(tc.tile_pool(name="vo", bufs=2))
                vstage_pool = es_attn.enter_context(tc.tile_pool(name="vstage", bufs=2))
                exp_pool = es_attn.enter_context(tc.tile_pool(name="exp", bufs=3))
                rsb_pool = es_attn.enter_context(tc.tile_pool(name="rsb", bufs=2))

                xts = []
                for ct in range(CT_TILES):
                    t = xt_pool.tile([128, NQ], BF16, tag="xt", name=f"xts{ct}")
                    nc.sync.dma_start(out=t[:], in_=xt[ct * 128:(ct + 1) * 128, :])
                    xts.append(t)

                def prepare_qkv(p):
                    """Allocate tiles + emit thunks for pair p's QKV projection
                    over OWN tokens, plus the pairwise K/V AllGather."""
                    w_sb = wq_pool.tile([128, 3 * CT_TILES * 128], BF16,
                                        tag="w", name=f"w{p}")
                    for ct in range(CT_TILES):
                        nc.sync.dma_start(
                            out=w_sb[:, ct * 384:(ct + 1) * 384],
                            in_=wqkvt[ct * 128:(ct + 1) * 128,
                                      p * 384:(p + 1) * 384],
                        )

                    def w_slice(m, ct):
                        o = ct * 384 + m * 128
                        return w_sb[:, o:o + 128]

                    qt_sb = qt_pool.tile([128, NQ], BF16, tag="qt", name=f"qt{p}")
                    kt_sb = kt_pool.tile([128, NK], BF16, tag="kt", name=f"kt{p}")
                    kown = ko_pool.tile([128, NQ], BF16, tag="kown", name=f"kown{p}")
                    vown = vo_pool.tile([128, KT_OWN * SLOTW], BF16, tag="vown",
                                        name=f"vown{p}")
                    vp = vprimes[p % 2]
                    thunks = []

                    def k_chunk(tch):
                        def f():
                            ps = psum_mm.tile([128, 512], F32, tag="mm", name="psk")
                            for ct in range(CT_TILES):
                                nc.tensor.matmul(
                                    ps[:], w_slice(1, ct),
                                    xts[ct][:, tch * 512:(tch + 1) * 512],
                                    start=(ct == 0), stop=(ct == CT_TILES - 1),
                                )
                            nc.vector.tensor_copy(
                                kown[:, tch * 512:(tch + 1) * 512], ps[:])
                        return f

                    def v_chunk(tch):
                        def f():
                            if tch == 0:
                                # ones columns for the softmax denominators;
                                # the 64-wide copies below leave them intact
                                nc.gpsimd.memset(vown[:], 1.0)
                            ps = psum_mm.tile([128, 512], F32, tag="mm", name="psv")
                            for ct in range(CT_TILES):
                                nc.tensor.matmul(
                                    ps[:], w_slice(2, ct),
                                    xts[ct][:, tch * 512:(tch + 1) * 512],
                                    start=(ct == 0), stop=(ct == CT_TILES - 1),
                                )
                            vs = vstage_pool.tile([128, 512], BF16, tag="vs")
                            nc.vector.tensor_copy(vs[:], ps[:])
                            for sub in range(4):
                                kt_idx = tch * 4 + sub
                                pt = psum_mm.tile([128, 512], BF16,
                                                  tag="mm", name="pt")
                                nc.tensor.matmul(
                                    pt[:, 0:128],
                                    vs[:, sub * 128:(sub + 1) * 128],
                                    ident[:],
                                    is_transpose=True,
                                )
                                so = kt_idx * SLOTW
                                nc.vector.tensor_copy(
                                    vown[:, so:so + HD], pt[:, 0:HD])
                                nc.vector.tensor_copy(
                                    vown[:, so + VSLOT:so + VSLOT + HD],
                                    pt[:, HD:2 * HD])
                        return f

                    def bounce_thunk():
                        nc.gpsimd.dma_start(out=b_ins[p][:, 0:NQ], in_=kown[:])
                        nc.gpsimd.dma_start(out=b_ins[p][:, NQ:BOUNCE_W],
                                            in_=vown[:])

                    def cc_thunk():
                        nc.gpsimd.collective_compute(
                            "AllGather",
                            mybir.AluOpType.bypass,
                            replica_groups=groups,
                            ins=[b_ins[p].opt()],
                            outs=[b_outs[p].opt()],
                        )

                    def read_thunk(s):
                        def f():
                            nc.sync.dma_start(
                                out=kt_sb[:, s * NQ:(s + 1) * NQ],
                                in_=b_outs[p][s][:, 0:NQ])
                            nc.sync.dma_start(
                                out=vp[:, s * KT_OWN * SLOTW:
                                       (s + 1) * KT_OWN * SLOTW],
                                in_=b_outs[p][s][:, NQ:BOUNCE_W])
                        return f

                    def q_chunk(tch):
                        def f():
                            ps = psum_mm.tile([128, 512], F32, tag="mm", name="psq")
                            for ct in range(CT_TILES):
                                nc.tensor.matmul(
                                    ps[:], w_slice(0, ct),
                                    xts[ct][:, tch * 512:(tch + 1) * 512],
                                    start=(ct == 0), stop=(ct == CT_TILES - 1),
                                )
                            nc.vector.tensor_copy(
                                qt_sb[:, tch * 512:(tch + 1) * 512], ps[:])
                        return f

                    for tch in range(NQ // 512):
                        thunks.append(k_chunk(tch))
                    for tch in range(NQ // 512):
                        thunks.append(v_chunk(tch))
                    thunks.append(bounce_thunk)
                    thunks.append(cc_thunk)
                    thunks.append(read_thunk(0))
                    thunks.append(read_thunk(1))
                    for tch in range(NQ // 512):
                        thunks.append(q_chunk(tch))
                    return qt_sb, kt_sb, vp, thunks

                # prologue: pair 0 QKV + exchange fully
                qt_sb, kt_sb, vp_cur, thunks = prepare_qkv(0)
                for t in thunks:
                    t()

                def proj_batch_thunks(pairs, with_bias):
                    """Accumulate nts[a] (+ nts[b] (+ bias)) into oaccs via one
                    PSUM group per (tt, oc) tile."""
                    thunks = []
                    first = with_bias  # bias batch is also the first write
                    for tt in range(NQ // 128):
                        for oc in range(C // 512):
                            def f(tt=tt, oc=oc):
                                po = psum_mm.tile([128, 512], F32, tag="mm",
                                                  name="pp")
                                n = len(pairs) + (1 if with_bias else 0)
                                i = 0
                                if with_bias:
                                    nc.tensor.matmul(
                                        po[:], ones[0:1, :],
                                        bias_sb[0:1, oc * 512:(oc + 1) * 512],
                                        start=True, stop=(n == 1),
                                    )
                                    i = 1
                                for pr in pairs:
                                    nc.tensor.matmul(
                                        po[:],
                                        nts[pr][:, tt * 128:(tt + 1) * 128],
                                        wpts[pr][:, oc * 512:(oc + 1) * 512],
                                        start=(i == 0), stop=(i == n - 1),
                                    )
                                    i += 1
                                osl = oaccs[tt][:, oc * 512:(oc + 1) * 512]
                                if first:
                                    nc.vector.tensor_copy(osl, po[:])
                                else:
                                    nc.vector.tensor_add(out=osl, in0=osl,
                                                         in1=po[:])
                            thunks.append(f)
                    return thunks

                for p in range(NPAIR):
                    if p == 1:
                        for ct in range(CT_TILES):
                            nc.sync.dma_start(
                                out=wpts[ct][:],
                                in_=wpt[ct * 128:(ct + 1) * 128, :])
                    if p + 1 < NPAIR:
                        nqt, nkt, nvp, nthunks = prepare_qkv(p + 1)
                    else:
                        nqt = nkt = nvp = None
                        nthunks = []
                    if p == 2:
                        nthunks = nthunks + proj_batch_thunks([0, 1], True)
                    elif p == 4:
                        nthunks = nthunks + proj_batch_thunks([2, 3], False)
                    elif p == 6:
                        nthunks = nthunks + proj_batch_thunks([4, 5], False)
                    elif p == 7:
                        nthunks = nthunks + proj_batch_thunks([6], False)
                    n_slots = 2 * KT_TILES
                    sched = {}
                    for i, t in enumerate(nthunks):
                        sched.setdefault(
                            min(n_slots - 1, (i * n_slots) // max(1, len(nthunks))),
                            []).append(t)

                    with nc.named_scope(f"attn{p}"):
                        slot_i = 0
                        for h2 in range(2):
                            rb = h2 * 64
                            uacc = psum_u.tile([65, NQ], F32, tag="u",
                                               name=f"uacc{h2}")
                            for kt_idx in range(KT_TILES):
                                ps = psum_s.tile([128, NQ], F32, tag="s", name="pss")
                                ko = kt_idx * 128
                                for qc in range(NQ // 512):
                                    qsl = slice(qc * 512, (qc + 1) * 512)
                                    nc.tensor.matmul(
                                        ps[:, qsl],
                                        kt_sb[rb:rb + 64, ko:ko + 128],
                                        qt_sb[rb:rb + 64, qsl],
                                    )
                                esb = exp_pool.tile([128, NQ], BF16, tag="e")
                                nc.scalar.activation(
                                    esb[:], ps[:],
                                    mybir.ActivationFunctionType.Exp, scale=SCALE)
                                slot = kt_idx * SLOTW + h2 * VSLOT
                                for qc in range(NQ // 512):
                                    qsl = slice(qc * 512, (qc + 1) * 512)
                                    nc.tensor.matmul(
                                        uacc[:, qsl],
                                        vp_cur[:, slot:slot + VSLOT],
                                        esb[:, qsl],
                                        start=(kt_idx == 0),
                                        stop=(kt_idx == KT_TILES - 1),
                                    )
                                for t in sched.get(slot_i, []):
                                    t()
                                slot_i += 1

                            # normalization for head h2
                            nt = nts[p]
                            stg = rsb_pool.tile([65, NQ], BF16, tag="stg", name="stg")
                            nc.vector.tensor_copy(stg[:], uacc[:])
                            t8 = rsb_pool.tile([8, NQ // 8], BF16, tag="t8", name="t8")
                            nc.gpsimd.dma_start(out=t8[:], in_=stg[64:65, :])
                            r8 = rsb_pool.tile([8, NQ // 8], BF16, tag="r8", name="r8")
                            with nc.allow_low_precision("bf16 matmul operand"):
                                nc.vector.reciprocal(r8[:], t8[:])
                            rsb = rsb_pool.tile([1, NQ], BF16, tag="r")
                            nc.gpsimd.dma_start(out=rsb[:], in_=r8[:])
                            nc.vector.tensor_copy(nt[rb:rb + 64, :], stg[0:64, :])
                            for qc in range(NQ // 512):
                                pb = psum_mm.tile([128, 512], F32, tag="mm", name="pb")
                                nc.tensor.matmul(
                                    pb[0:64, :], ones[0:1, 0:64],
                                    rsb[0:1, qc * 512:(qc + 1) * 512],
                                )
                                nc.vector.tensor_mul(
                                    out=nt[rb:rb + 64, qc * 512:(qc + 1) * 512],
                                    in0=nt[rb:rb + 64, qc * 512:(qc + 1) * 512],
                                    in1=pb[0:64, :],
                                )
                    qt_sb, kt_sb, vp_cur = nqt, nkt, nvp

            with contextlib.ExitStack() as es_proj:
                out_pool = es_proj.enter_context(tc.tile_pool(name="outp", bufs=3))
                with nc.named_scope("proj"):
                    for tt in range(NQ // 128):
                        for oc in range(C // 512):
                            po = psum_mm.tile([128, 512], F32, tag="mm", name="po")
                            nc.tensor.matmul(
                                po[:],
                                nts[NPAIR - 1][:, tt * 128:(tt + 1) * 128],
                                wpts[NPAIR - 1][:, oc * 512:(oc + 1) * 512],
                            )
                            ob = out_pool.tile([128, 512], F32, tag="ob")
                            nc.vector.tensor_add(
                                out=ob[:],
                                in0=oaccs[tt][:, oc * 512:(oc + 1) * 512],
                                in1=po[:],
                            )
                            nc.sync.dma_start(
                                out=out[tt * 128:(tt + 1) * 128,
                                        oc * 512:(oc + 1) * 512],
                                in_=ob[:],
                            )
    return nc


def make_in_maps(x, w_qkv, w_proj, b_proj):
    import ml_dtypes
    bf16 = ml_dtypes.bfloat16
    # reorder w_qkv rows [3, pair, 128] and transpose -> [c, (pair, m, d)]
    wq = np.asarray(w_qkv).reshape(3, NPAIR, 128, C)
    wqkvt = np.ascontiguousarray(
        wq.transpose(3, 1, 0, 2).reshape(C, 3 * C).astype(bf16))
    wpt = np.ascontiguousarray(np.asarray(w_proj).T.astype(bf16))
    bias = np.ascontiguousarray(np.asarray(b_proj).reshape(1, C).astype(bf16))
    in_maps = []
    for c in range(8):
        b, qh = c // 2, c % 2
        xown = np.asarray(x)[b, qh * NQ:(qh + 1) * NQ]
        xt = np.ascontiguousarray(xown.T.astype(bf16))
        in_maps.append({"xt": xt, "wqkvt": wqkvt, "wpt": wpt, "bias": bias})
    return in_maps


def assemble_output(results, x_shape):
    B, N, Cm = x_shape
    outp = np.empty((B, N, Cm), dtype=np.float32)
    for c in range(8):
        b, qh = c // 2, c % 2
        outp[b, qh * NQ:(qh + 1) * NQ, :] = results[c]["out"]
    return outp


_nc_cache = []


def kernel(x, w_qkv, w_proj, b_proj):
    from concourse.bass_utils import run_bass_kernel_spmd

    _apply_patches()
    x = np.asarray(x)
    if not _nc_cache:
        _nc_cache.append(build_nc())
    nc = _nc_cache[0]
    in_maps = make_in_maps(x, np.asarray(w_qkv), np.asarray(w_proj),
                           np.asarray(b_proj))
    res = run_bass_kernel_spmd(nc, in_maps, core_ids=list(range(8)))
    return assemble_output(res.results, (4, 2048, 1024)).astype(np.float32)


# revision 11
# speedup vs baseline: 1.0202x; 1.0202x over previous
"""nn_Attention multi-head attention on 8 TRN2 NeuronCores.

Sharding: core c handles batch b=c//2 and query-half qh=c%2 (1024 query
tokens). QKV projections run only over the core's OWN 1024 tokens; the
K^T/V halves are exchanged between the two cores of a batch with a
pairwise HBM AllGather (replica groups {2b, 2b+1}), so no projection
work is duplicated. Keys are kept in global batch order (even core's
tokens first) on both cores, so the exchange read-back is
program-uniform. The host concatenates the 8 disjoint [1024, 1024]
output slices.

Device-side structure (per core, per head-pair p):
  - w_qkv slices loaded with one DMA per contraction tile from a
    host-reordered [c, (pair, m, d)] layout
  - K^T/V'/Q^T computed for own tokens; V' obtained with full [128,128]
    PE transposes (both heads at once) into 65-wide V-slots whose last
    column is 1.0 (softmax denominator via the ones column)
  - own K^T/V' halves staged to an HBM bounce, AllGather'd across the
    core pair, and read back into the full-key kt_sb / vp buffers
  - attention in transposed layout S^T = K_h Q_h^T per 128-key tile;
    exp on the Scalar engine straight out of PSUM; all matmuls bf16
    with fp32 PSUM
  - the QKV+exchange of the NEXT pair and the output-projection
    partials of PREVIOUS pairs (batched two pairs per PSUM
    accumulation group, bias folded into the first batch) are
    emission-interleaved into the attention key-tile loop
"""

import contextlib

import numpy as np
import orjson

import concourse.bass as bass
import concourse.mybir as mybir
import concourse.tile as tile
from concourse.vector_clock import ScopedClock

# ---------------------------------------------------------------------------
# Workarounds for the walrus build in this container, which accepts at most
# one sync wait per engine instruction (two for EventSemaphore):
#  1. Tile's end-of-kernel drain carries one wait per outstanding semaphore --
#     redistribute over a chain of sync-engine NOPs.
#  2. Tile's scheduler also emits multi-wait body instructions -- split them
#     in the serialized BIR by inserting same-engine NOPs ahead of the
#     offender (engine program order makes the chain equivalent).
# ---------------------------------------------------------------------------


def _patched_drain_and_barrier(self, tick_clock, wait_clock):
    nc = self.nc
    collector = nc.sync.nop()
    wait_clock.add_sem_waits(
        collector.ins, ScopedClock({None: tick_clock.global_clock})
    )
    si = collector.ins.sync_info
    waits = list(si.on_wait or []) if si is not None else []
    if si is not None:
        si.on_wait = waits[:1]
    import bass_rust as _br

    for w in waits[1:]:
        n = nc.sync.nop()
        n.ins.sync_info = _br.SyncInfo(on_wait=[w], on_update=[])

    nc.sync.drain()
    nc.all_engine_barrier()
    assert self.sems is not None
    popped = nc._tile_sem_poison_stack.pop()
    assert popped is self._sem_poison
    nc.clear_and_free_semaphores(list(self.sems.allocated().values()))
    nc.all_engine_barrier()


_WCAPS = {"EventSemaphore": 2}
_wcounter = [0]


def _split_waits_json(bir_bytes: bytes) -> bytes:
    j = orjson.loads(bir_bytes)
    changed_any = False
    for f in j.get("functions", []):
        for b in f.get("blocks", []):
            outl = []
            changed = False
            for ins in b["instructions"]:
                si = ins.get("sync_info")
                waits = (si or {}).get("on_wait") or []
                cap = _WCAPS.get(ins.get("opcode"), 1)
                engine = ins.get("engine")
                if len(waits) > cap and engine and engine != "Unassigned":
                    changed = True
                    extra, keep = waits[:-cap], waits[-cap:]
                    for w in extra:
                        _wcounter[0] += 1
                        outl.append({
                            "name": f"I-wsplit-{_wcounter[0]}",
                            "opcode": "NoOp",
                            "engine": engine,
                            "ins": [],
                            "outs": [],
                            "sync_info": {"on_update": [], "on_wait": [w]},
                        })
                    si["on_wait"] = keep
                outl.append(ins)
            if changed:
                b["instructions"] = outl
                changed_any = True
    return orjson.dumps(j) if changed_any else bir_bytes


def _apply_patches():
    if not getattr(tile.TileContext, "_attn_drain_patched", False):
        tile.TileContext._drain_and_barrier = _patched_drain_and_barrier
        tile.TileContext._attn_drain_patched = True
    if not getattr(bass.Bass, "_attn_wait_split_patched", False):
        orig = bass.Bass.to_json_bytes

        def to_json_bytes(self, *a, **kw):
            return _split_waits_json(orig(self, *a, **kw))

        bass.Bass.to_json_bytes = to_json_bytes
        bass.Bass._attn_wait_split_patched = True


F32 = mybir.dt.float32
BF16 = mybir.dt.bfloat16

C = 1024
H = 16
HD = 64
NK = 2048
NQ = 1024
SCALE = HD ** -0.5
KT_TILES = NK // 128   # 16 key tiles (full)
KT_OWN = NQ // 128     # 8 key tiles computed locally
CT_TILES = C // 128
VSLOT = 65             # 64 v dims + ones column
SLOTW = 2 * VSLOT      # both heads of a pair per key tile
NPAIR = H // 2
BOUNCE_W = NQ + KT_OWN * SLOTW  # 1024 K cols + 1040 V cols


def build_nc():
    _apply_patches()
    nc = bass.Bass("TRN2", num_devices=8)
    xt = nc.declare_dram_parameter("xt", [C, NQ], BF16, isOutput=False)
    wqkvt = nc.declare_dram_parameter("wqkvt", [C, 3 * C], BF16, isOutput=False)
    wpt = nc.declare_dram_parameter("wpt", [C, C], BF16, isOutput=False)
    bias = nc.declare_dram_parameter("bias", [1, C], BF16, isOutput=False)
    out = nc.declare_dram_parameter("out", [NQ, C], F32, isOutput=True)

    groups = [[0, 1], [2, 3], [4, 5], [6, 7]]

    with tile.TileContext(nc) as tc:
        with contextlib.ExitStack() as es:
            persist = es.enter_context(tc.tile_pool(name="persist", bufs=1))
            ones = persist.tile([1, 128], BF16, tag="ones")
            nc.vector.memset(ones[:], 1.0)
            ident = persist.tile([128, 128], BF16, tag="ident")
            nc.gpsimd.memset(ident[:], 0.0)
            nc.gpsimd.affine_select(
                out=ident[:], in_=ident[:],
                compare_op=mybir.AluOpType.not_equal, fill=1.0,
                base=0, pattern=[[-1, 128]], channel_multiplier=1,
            )
            bias_sb = persist.tile([1, C], BF16, tag="bias")
            nc.sync.dma_start(out=bias_sb[:], in_=bias[:])

            nts = [persist.tile([128, NQ], BF16, tag=f"nt{ct}", name=f"nt{ct}")
                   for ct in range(CT_TILES)]
            vprimes = [persist.tile([128, KT_TILES * SLOTW], BF16,
                                    tag=f"vp{i}", name=f"vp{i}") for i in range(2)]
            oaccs = [persist.tile([128, C], F32, tag=f"oa{tt}", name=f"oa{tt}")
                     for tt in range(NQ // 128)]
            wp_pool = es.enter_context(tc.tile_pool(name="wp", bufs=CT_TILES))
            wpts = [wp_pool.tile([128, C], BF16, tag="wpt", name=f"wpts{ct}")
                    for ct in range(CT_TILES)]

            dram = es.enter_context(tc.tile_pool(name="dram", bufs=1, space="DRAM"))
            b_ins = [dram.tile([128, BOUNCE_W], BF16, tag=f"bi{p}", name=f"bi{p}")
                     for p in range(NPAIR)]
            b_outs = [dram.tile([2, 128, BOUNCE_W], BF16, tag=f"bo{p}",
                                name=f"bo{p}") for p in range(NPAIR)]

            psum_mm = es.enter_context(tc.tile_pool(name="psum_mm", bufs=2, space="PSUM"))
            psum_s = es.enter_context(tc.tile_pool(name="psum_s", bufs=2, space="PSUM"))
            psum_u = es.enter_context(tc.tile_pool(name="psum_u", bufs=1, space="PSUM"))

            with contextlib.ExitStack() as es_attn:
                xt_pool = es_attn.enter_context(tc.tile_pool(name="xtp", bufs=CT_TILES))
                wq_pool = es_attn.enter_context(tc.tile_pool(name="wq", bufs=2))
                kt_pool = es_attn.enter_context(tc.tile_pool(name="kt", bufs=2))
                qt_pool = es_attn.enter_context(tc.tile_pool(name="qt", bufs=2))
                ko_pool = es_attn.enter_context(tc.tile_pool(name="ko", bufs=2))
                vo_pool = es_attn.enter_context(tc.tile_pool(name="vo", bufs=2))
                vstage_pool = es_attn.enter_context(tc.tile_pool(name="vstage", bufs=2))
                exp_pool = es_attn.enter_context(tc.tile_pool(name="exp", bufs=3))
                rsb_pool = es_attn.enter_context(tc.tile_pool(name="rsb", bufs=2))

                xts = []
                for ct in range(CT_TILES):
                    t = xt_pool.tile([128, NQ], BF16, tag="xt", name=f"xts{ct}")
                    nc.sync.dma_start(out=t[:], in_=xt[ct * 128:(ct + 1) * 128, :])
                    xts.append(t)

                def prepare_qkv(p):
                    """Allocate tiles + emit thunks for pair p's QKV projection
                    over OWN tokens, plus the pairwise K/V AllGather."""
                    w_sb = wq_pool.tile([128, 3 * CT_TILES * 128], BF16,
                                        tag="w", name=f"w{p}")
                    for ct in range(CT_TILES):
                        nc.sync.dma_start(
                            out=w_sb[:, ct * 384:(ct + 1) * 384],
                            in_=wqkvt[ct * 128:(ct + 1) * 128,
                                      p * 384:(p + 1) * 384],
                        )

                    def w_slice(m, ct):
                        o = ct * 384 + m * 128
                        return w_sb[:, o:o + 128]

                    qt_sb = qt_pool.tile([128, NQ], BF16, tag="qt", name=f"qt{p}")
                    kt_sb = kt_pool.tile([128, NK], BF16, tag="kt", name=f"kt{p}")
                    kown = ko_pool.tile([128, NQ], BF16, tag="kown", name=f"kown{p}")
                    vown = vo_pool.tile([128, KT_OWN * SLOTW], BF16, tag="vown",
                                        name=f"vown{p}")
                    vp = vprimes[p % 2]
                    thunks = []

                    def k_chunk(tch):
                        def f():
                            ps = psum_mm.tile([128, 512], F32, tag="mm", name="psk")
                            for ct in range(CT_TILES):
                                nc.tensor.matmul(
                                    ps[:], w_slice(1, ct),
                                    xts[ct][:, tch * 512:(tch + 1) * 512],
                                    start=(ct == 0), stop=(ct == CT_TILES - 1),
                                )
                            nc.vector.tensor_copy(
                                kown[:, tch * 512:(tch + 1) * 512], ps[:])
                        return f

                    def v_chunk(tch):
                        def f():
                            if tch == 0:
                                # ones columns for the softmax denominators;
                                # the 64-wide copies below leave them intact
                                nc.gpsimd.memset(vown[:], 1.0)
                            ps = psum_mm.tile([128, 512], F32, tag="mm", name="psv")
                            for ct in range(CT_TILES):
                                nc.tensor.matmul(
                                    ps[:], w_slice(2, ct),
                                    xts[ct][:, tch * 512:(tch + 1) * 512],
                                    start=(ct == 0), stop=(ct == CT_TILES - 1),
                                )
                            vs = vstage_pool.tile([128, 512], BF16, tag="vs")
                            nc.vector.tensor_copy(vs[:], ps[:])
                            for sub in range(4):
                                kt_idx = tch * 4 + sub
                                pt = psum_mm.tile([128, 512], BF16,
                                                  tag="mm", name="pt")
                                nc.tensor.matmul(
                                    pt[:, 0:128],
                                    vs[:, sub * 128:(sub + 1) * 128],
                                    ident[:],
                                    is_transpose=True,
                                )
                                so = kt_idx * SLOTW
                                nc.vector.tensor_copy(
                                    vown[:, so:so + HD], pt[:, 0:HD])
                                nc.vector.tensor_copy(
                                    vown[:, so + VSLOT:so + VSLOT + HD],
                                    pt[:, HD:2 * HD])
                        return f

                    def bounce_thunk():
                        nc.sync.dma_start(out=b_ins[p][:, 0:NQ], in_=kown[:])
                        nc.sync.dma_start(out=b_ins[p][:, NQ:BOUNCE_W],
                                          in_=vown[:])

                    def cc_thunk():
                        nc.gpsimd.collective_compute(
                            "AllGather",
                            mybir.AluOpType.bypass,
                            replica_groups=groups,
                            ins=[b_ins[p].opt()],
                            outs=[b_outs[p].opt()],
                        )

                    def read_thunk(s):
                        def f():
                            nc.sync.dma_start(
                                out=kt_sb[:, s * NQ:(s + 1) * NQ],
                                in_=b_outs[p][s][:, 0:NQ])
                            nc.sync.dma_start(
                                out=vp[:, s * KT_OWN * SLOTW:
                                       (s + 1) * KT_OWN * SLOTW],
                                in_=b_outs[p][s][:, NQ:BOUNCE_W])
                        return f

                    def q_chunk(tch):
                        def f():
                            ps = psum_mm.tile([128, 512], F32, tag="mm", name="psq")
                            for ct in range(CT_TILES):
                                nc.tensor.matmul(
                                    ps[:], w_slice(0, ct),
                                    xts[ct][:, tch * 512:(tch + 1) * 512],
                                    start=(ct == 0), stop=(ct == CT_TILES - 1),
                                )
                            nc.vector.tensor_copy(
                                qt_sb[:, tch * 512:(tch + 1) * 512], ps[:])
                        return f

                    # (slot, thunk) pairs: front-load K/V + the exchange so the
                    # collective fires early in the previous pair's attention;
                    # Q and the read-back have plenty of slack.
                    slotted = [
                        (0, k_chunk(0)), (2, k_chunk(1)),
                        (4, v_chunk(0)), (6, v_chunk(1)),
                        (7, bounce_thunk), (8, cc_thunk),
                        (11, read_thunk(0)), (13, read_thunk(1)),
                        (16, q_chunk(0)), (19, q_chunk(1)),
                    ]
                    return qt_sb, kt_sb, vp, slotted

                def make_norm_finish(p, h2, stg):
                    """Deferred tail of the softmax normalization: reciprocal
                    of the denominators and the nt scale, interleaved into a
                    later attention window so the PE never waits on it."""
                    nt = nts[p]
                    rb = h2 * 64

                    def f():
                        t8 = rsb_pool.tile([8, NQ // 8], BF16, tag="t8",
                                           name="t8")
                        nc.gpsimd.dma_start(out=t8[:], in_=stg[64:65, :])
                        r8 = rsb_pool.tile([8, NQ // 8], BF16, tag="r8",
                                           name="r8")
                        with nc.allow_low_precision("bf16 matmul operand"):
                            nc.vector.reciprocal(r8[:], t8[:])
                        rsb = rsb_pool.tile([1, NQ], BF16, tag="r")
                        nc.gpsimd.dma_start(out=rsb[:], in_=r8[:])
                        nc.vector.tensor_copy(nt[rb:rb + 64, :], stg[0:64, :])
                        for qc in range(NQ // 512):
                            pb = psum_mm.tile([128, 512], F32, tag="mm",
                                              name="pb")
                            nc.tensor.matmul(
                                pb[0:64, :], ones[0:1, 0:64],
                                rsb[0:1, qc * 512:(qc + 1) * 512],
                            )
                            nc.vector.tensor_mul(
                                out=nt[rb:rb + 64, qc * 512:(qc + 1) * 512],
                                in0=nt[rb:rb + 64, qc * 512:(qc + 1) * 512],
                                in1=pb[0:64, :],
                            )
                    return f

                # prologue: pair 0 QKV + exchange; the collective is hidden
                # behind pair 0's Q chunks and pair 1's K chunks
                qt_sb, kt_sb, vp_cur, slotted0 = prepare_qkv(0)
                t0 = [t for _, t in slotted0]
                for i in (0, 1, 2, 3, 4, 8, 9):  # k,k,v,v,bounce,q,q
                    t0[i]()
                pending_reads0 = [t0[5], t0[6], t0[7]]  # cc, read, read

                def proj_batch_thunks(pairs, with_bias):
                    """Accumulate nts[a] (+ nts[b] (+ bias)) into oaccs via one
                    PSUM group per (tt, oc) tile."""
                    thunks = []
                    first = with_bias  # bias batch is also the first write
                    for tt in range(NQ // 128):
                        for oc in range(C // 512):
                            def f(tt=tt, oc=oc):
                                po = psum_mm.tile([128, 512], F32, tag="mm",
                                                  name="pp")
                                n = len(pairs) + (1 if with_bias else 0)
                                i = 0
                                if with_bias:
                                    nc.tensor.matmul(
                                        po[:], ones[0:1, :],
                                        bias_sb[0:1, oc * 512:(oc + 1) * 512],
                                        start=True, stop=(n == 1),
                                    )
                                    i = 1
                                for pr in pairs:
                                    nc.tensor.matmul(
                                        po[:],
                                        nts[pr][:, tt * 128:(tt + 1) * 128],
                                        wpts[pr][:, oc * 512:(oc + 1) * 512],
                                        start=(i == 0), stop=(i == n - 1),
                                    )
                                    i += 1
                                osl = oaccs[tt][:, oc * 512:(oc + 1) * 512]
                                if first:
                                    nc.vector.tensor_copy(osl, po[:])
                                else:
                                    nc.vector.tensor_add(out=osl, in0=osl,
                                                         in1=po[:])
                            thunks.append(f)
                    return thunks

                pending_norm = []
                for p in range(NPAIR):
                    if p == 1:
                        for ct in range(CT_TILES):
                            nc.sync.dma_start(
                                out=wpts[ct][:],
                                in_=wpt[ct * 128:(ct + 1) * 128, :])
                    if p + 1 < NPAIR:
                        nqt, nkt, nvp, slotted = prepare_qkv(p + 1)
                    else:
                        nqt = nkt = nvp = None
                        slotted = []
                    if p == 0:
                        # pair-0 collective after pair-1's weight DMAs, hidden
                        # behind pair-1 K work; then pair 0's read-back
                        pending_reads0[0]()   # cc0
                        slotted[0][1]()       # pair-1 k0
                        slotted[1][1]()       # pair-1 k1
                        pending_reads0[1]()
                        pending_reads0[2]()
                        slotted = slotted[2:]
                    sched = {}
                    for i, t in enumerate(pending_norm):
                        sched.setdefault(i, []).append(t)
                    pending_norm = []
                    for s, t in slotted:
                        sched.setdefault(s, []).append(t)
                    if p == 2:
                        pt_thunks = proj_batch_thunks([0, 1], True)
                    elif p == 4:
                        pt_thunks = proj_batch_thunks([2, 3], False)
                    elif p == 6:
                        pt_thunks = proj_batch_thunks([4, 5], False)
                    elif p == 7:
                        pt_thunks = proj_batch_thunks([6], False)
                    else:
                        pt_thunks = []
                    for i, t in enumerate(pt_thunks):
                        sched.setdefault(
                            13 + (i * 19) // max(1, len(pt_thunks)), []).append(t)

                    with nc.named_scope(f"attn{p}"):
                        slot_i = 0
                        for h2 in range(2):
                            rb = h2 * 64
                            uacc = psum_u.tile([65, NQ], F32, tag="u",
                                               name=f"uacc{h2}")
                            for kt_idx in range(KT_TILES):
                                ps = psum_s.tile([128, NQ], F32, tag="s", name="pss")
                                ko = kt_idx * 128
                                for qc in range(NQ // 512):
                                    qsl = slice(qc * 512, (qc + 1) * 512)
                                    nc.tensor.matmul(
                                        ps[:, qsl],
                                        kt_sb[rb:rb + 64, ko:ko + 128],
                                        qt_sb[rb:rb + 64, qsl],
                                    )
                                esb = exp_pool.tile([128, NQ], BF16, tag="e")
                                nc.scalar.activation(
                                    esb[:], ps[:],
                                    mybir.ActivationFunctionType.Exp, scale=SCALE)
                                slot = kt_idx * SLOTW + h2 * VSLOT
                                for qc in range(NQ // 512):
                                    qsl = slice(qc * 512, (qc + 1) * 512)
                                    nc.tensor.matmul(
                                        uacc[:, qsl],
                                        vp_cur[:, slot:slot + VSLOT],
                                        esb[:, qsl],
                                        start=(kt_idx == 0),
                                        stop=(kt_idx == KT_TILES - 1),
                                    )
                                for t in sched.get(slot_i, []):
                                    t()
                                slot_i += 1

                            # drain the AV accumulator; the rest of the
                            # normalization is deferred into later slots
                            stg = rsb_pool.tile([65, NQ], BF16, tag="stg",
                                                name="stg")
                            nc.vector.tensor_copy(stg[:], uacc[:])
                            fin = make_norm_finish(p, h2, stg)
                            if h2 == 0:
                                sched.setdefault(17, []).append(fin)
                            elif p + 1 < NPAIR:
                                pending_norm.append(fin)
                            else:
                                fin()
                    qt_sb, kt_sb, vp_cur = nqt, nkt, nvp

            with contextlib.ExitStack() as es_proj:
                out_pool = es_proj.enter_context(tc.tile_pool(name="outp", bufs=3))
                with nc.named_scope("proj"):
                    for tt in range(NQ // 128):
                        for oc in range(C // 512):
                            po = psum_mm.tile([128, 512], F32, tag="mm", name="po")
                            nc.tensor.matmul(
                                po[:],
                                nts[NPAIR - 1][:, tt * 128:(tt + 1) * 128],
                                wpts[NPAIR - 1][:, oc * 512:(oc + 1) * 512],
                            )
                            ob = out_pool.tile([128, 512], F32, tag="ob")
                            nc.vector.tensor_add(
                                out=ob[:],
                                in0=oaccs[tt][:, oc * 512:(oc + 1) * 512],
                                in1=po[:],
                            )
                            nc.sync.dma_start(
                                out=out[tt * 128:(tt + 1) * 128,
                                        oc * 512:(oc + 1) * 512],
                                in_=ob[:],
                            )
    return nc


def make_in_maps(x, w_qkv, w_proj, b_proj):
    import ml_dtypes
    bf16 = ml_dtypes.bfloat16
    # reorder w_qkv rows [3, pair, 128] and transpose -> [c, (pair, m, d)]
    wq = np.asarray(w_qkv).reshape(3, NPAIR, 128, C)
    wqkvt = np.ascontiguousarray(
        wq.transpose(3, 1, 0, 2).reshape(C, 3 * C).astype(bf16))
    wpt = np.ascontiguousarray(np.asarray(w_proj).T.astype(bf16))
    bias = np.ascontiguousarray(np.asarray(b_proj).reshape(1, C).astype(bf16))
    in_maps = []
    for c in range(8):
        b, qh = c // 2, c % 2
        xown = np.asarray(x)[b, qh * NQ:(qh + 1) * NQ]
        xt = np.ascontiguousarray(xown.T.astype(bf16))
        in_maps.append({"xt": xt, "wqkvt": wqkvt, "wpt": wpt, "bias": bias})
    return in_maps


def assemble_output(results, x_shape):
    B, N, Cm = x_shape
    outp = np.empty((B, N, Cm), dtype=np.float32)
    for c in range(8):
        b, qh = c // 2, c % 2
        outp[b, qh * NQ:(qh + 1) * NQ, :] = results[c]["out"]
    return outp


_nc_cache = []


def kernel(x, w_qkv, w_proj, b_proj):
    from concourse.bass_utils import run_bass_kernel_spmd

    _apply_patches()
    x = np.asarray(x)
    if not _nc_cache:
        _nc_cache.append(build_nc())
    nc = _nc_cache[0]
    in_maps = make_in_maps(x, np.asarray(w_qkv), np.asarray(w_proj),
                           np.asarray(b_proj))
    res = run_bass_kernel_spmd(nc, in_maps, core_ids=list(range(8)))
    return assemble_output(res.results, (4, 2048, 1024)).astype(np.float32)
